# revision 23
# baseline (speedup 1.0000x reference)
"""CRF decoder (projection + Viterbi decode + CRF log-likelihood loss) on 8 Trainium2 cores.

Strategy (data-parallel over batch, 4 batch elements per core):
 - host pre-transposes hiddens to [B, H, T] so the device can load [h, t] tiles
   contiguously; emission projection runs as fp32 PE matmuls (W chunks stationary).
 - forward (log-partition) scan runs in probability space: alpha_t in normalized
   prob form, transition mixing via one small bf16 PE matmul per step with a
   row-stochastic exp(transitions) matrix (row sums folded into the emission
   exponentials), periodic renormalization; log-corrections summed on host.
 - Viterbi max scan runs in fp32: per-step tensor_scalar candidate add (DVE+ACT),
   PE transposes, fused DVE max-reduce; the alpha history ships to DRAM and the
   (cheap, O(B*T*K)) backtrace runs on host, identical in exact arithmetic to the
   reference's backpointer path.
 - gold-path score and the final loss reduction are O(B*T) gathers on host.
"""

import os
import sys

for _p in ("/opt/trn_rl_repo", "/root/.axon_site/_ro/trn_rl_repo"):
    if os.path.isdir(_p) and _p not in sys.path:
        sys.path.append(_p)

from contextlib import ExitStack

import ml_dtypes
import numpy as np

B, T, H, K = 32, 512, 2048, 128
N_CORES = 8
BL = B // N_CORES          # batch per core
RENORM = 6                 # forward renorm interval (measure at t%R==0, apply at t%R==2)
N_STASH = (T - 3) // RENORM  # renorm measurements (t = R, 2R, ..., <= T-3)

_BUILT = None


def _build():
    import concourse.bass as bass
    import concourse.tile as tile
    from concourse import bacc, mybir

    f32 = mybir.dt.float32
    bf16 = mybir.dt.bfloat16
    AF = mybir.ActivationFunctionType
    OP = mybir.AluOpType

    nc = bacc.Bacc("TRN2", target_bir_lowering=False, debug=False, num_devices=N_CORES)

    # inputs
    hT = nc.dram_tensor("hT", [BL, H, T], f32, kind="ExternalInput").ap()
    Wm = nc.dram_tensor("Wm", [H, K], f32, kind="ExternalInput").ap()
    transM = nc.dram_tensor("transM", [K, K], f32, kind="ExternalInput").ap()
    EtilM = nc.dram_tensor("EtilM", [K, K], bf16, kind="ExternalInput").ap()
    identM = nc.dram_tensor("identM", [K, K], f32, kind="ExternalInput").ap()
    # bias columns: [K, 4] = (biasv, biaslnZ, pp0bias, v0bias)
    colsM = nc.dram_tensor("colsM", [K, 4], f32, kind="ExternalInput").ap()
    ones128M = nc.dram_tensor("ones128M", [K, 1], bf16, kind="ExternalInput").ap()
    ones1M = nc.dram_tensor("ones1M", [1, K], f32, kind="ExternalInput").ap()

    # outputs
    logits_out = nc.dram_tensor("logits_out", [K, T * BL], f32, kind="ExternalOutput").ap()
    hist_out = nc.dram_tensor("hist_out", [K, T * BL], f32, kind="ExternalOutput").ap()
    ppfin_out = nc.dram_tensor("ppfin_out", [K, BL], f32, kind="ExternalOutput").ap()
    stash_out = nc.dram_tensor("stash_out", [1, N_STASH * BL], f32, kind="ExternalOutput").ap()

    with tile.TileContext(nc) as tc, ExitStack() as ctx:
        const = ctx.enter_context(tc.tile_pool(name="const", bufs=1))
        big = ctx.enter_context(tc.tile_pool(name="big", bufs=1))

        w_sb = const.tile([K, H], f32)           # 16 W chunks, chunk h at cols [h*128,(h+1)*128)
        trans_sb = const.tile([K, K], f32)
        etil_sb = const.tile([K, K], bf16)
        ident_sb = const.tile([K, K], f32)
        cols_sb = const.tile([K, 4], f32)
        ones128_sb = const.tile([K, 1], bf16)
        ones1_sb = const.tile([1, K], f32)

        logits_sb = big.tile([K, T * BL], f32)   # col = t*BL + b
        eemz_sb = big.tile([K, T * BL], f32)     # exp(logits + lnZ), same layout
        hist_sb = big.tile([K, T * BL], f32)     # viterbi alpha history
        stash_sb = big.tile([1, N_STASH * BL], f32)
        pp0_sb = big.tile([K, BL], bf16)

        # W is [H, K] in DRAM; chunk h is rows [h*128,(h+1)*128) -> SBUF [128(h'), 128(k)]
        for h in range(H // K):
            nc.sync.dma_start(w_sb[:, h * K:(h + 1) * K], Wm[h * K:(h + 1) * K, :])
        nc.sync.dma_start(trans_sb[:], transM[:])
        nc.sync.dma_start(etil_sb[:], EtilM[:])
        nc.sync.dma_start(ident_sb[:], identM[:])
        nc.sync.dma_start(cols_sb[:], colsM[:])
        nc.sync.dma_start(ones128_sb[:], ones128M[:])
        nc.sync.dma_start(ones1_sb[:], ones1M[:])

        biasv = cols_sb[:, 0:1]
        biaslnZ = cols_sb[:, 1:2]
        pp0bias = cols_sb[:, 2:3]
        v0bias = cols_sb[:, 3:4]

        logits3 = logits_sb.rearrange("p (t b) -> p t b", b=BL)
        eemz3 = eemz_sb.rearrange("p (t b) -> p t b", b=BL)

        # ---------------- phase 1: emission projection ----------------
        with tc.tile_pool(name="hbuf", bufs=3) as hbuf, \
             tc.tile_pool(name="ppsum", bufs=2, space="PSUM") as ppsum:
            for b in range(BL):
                acc = ppsum.tile([K, T], f32)
                for h in range(H // K):
                    rhs = hbuf.tile([K, T], f32)
                    nc.sync.dma_start(rhs[:], hT[b, h * K:(h + 1) * K, :])
                    nc.tensor.matmul(acc[:], w_sb[:, h * K:(h + 1) * K], rhs[:],
                                     start=(h == 0), stop=(h == H // K - 1))
                # logits (+bias) into strided [:, :, b] columns
                nc.vector.tensor_scalar_add(logits3[:, :, b], acc[:], biasv)
                # exp(logits + lnZ) = exp(acc + bias + lnZ)
                nc.scalar.activation(eemz3[:, :, b], acc[:], AF.Exp, bias=biaslnZ)
                # forward init pp0 = exp(acc[:,0] + bias + start + lnZ)
                nc.scalar.activation(pp0_sb[:, b:b + 1], acc[:, 0:1], AF.Exp, bias=pp0bias)
                # viterbi init alpha0 = acc[:,0] + bias + start
                nc.vector.tensor_scalar_add(hist_sb[:, b:b + 1], acc[:, 0:1], v0bias)
        nc.sync.dma_start(logits_out[:], logits_sb[:])

        # -------- phase 2: fused forward scan (prob space) + viterbi max scan --------
        # forward: 1 chain, delayed renorm (measure at t%R==0, apply at t%R==2) so the
        #   renorm pipeline stays off the serial path.
        # viterbi: two independent half-phase pair-chains (lanes 01 / 23); per pair:
        #   DVE double-scalar cand + ACT two-stage cand, PE transpose-mode, own PSUM
        #   bank, own half-reduce writing alphaNoEmit into hist (host adds logits).
        with tc.tile_pool(name="ppp", bufs=3) as ppp, \
             tc.tile_pool(name="cand", bufs=3) as candp, \
             tc.tile_pool(name="af", bufs=3) as afp, \
             tc.tile_pool(name="ups", bufs=2, space="PSUM") as ups, \
             tc.tile_pool(name="rnps", bufs=1, space="PSUM") as rnps, \
             tc.tile_pool(name="ctps", bufs=2, space="PSUM") as ctps:
            pp = pp0_sb[:]
            af_prev = None
            stash_idx = 0
            pend_r = None
            # ~4us of normal-mode matmuls to flip the PE HAM clock gate to 8/8
            # before the scan; the forward's per-step matmul then keeps every
            # activity window non-idle so the PE never re-throttles.
            warm_ps = ctps.tile([K, 2 * K], f32, tag="ct0")
            for w in range(36):
                nc.tensor.matmul(warm_ps[:, 0:K], etil_sb[:], etil_sb[:],
                                 start=True, stop=True)
            for t in range(1, T):
                # ---- viterbi: two independent pair-chains, half-phase shifted ----
                # pair p covers lanes (2p, 2p+1): lane 2p cand on DVE (STT fuses ne+emit),
                # lane 2p+1 on ACT (ACT-side af then cand); each pair has its own PSUM
                # bank and its own half-reduce, so the chains never wait on each other.
                cand = candp.tile([K, BL * K], f32, tag="cand")
                for p in range(2):
                    if p == 1:
                        # tiny dummy normal-mode matmul mid-step: keeps the PE HAM
                        # activity window non-idle (transpose-mode alone reads as
                        # idle -> re-throttle to 1.2GHz)
                        wm = rnps.tile([1, 8], f32, tag="s")
                        nc.tensor.matmul(wm[:], etil_sb[:, 0:1], etil_sb[:, 0:8],
                                         start=True, stop=True)
                    ct = ctps.tile([K, 2 * K], f32, tag=f"ct{p}")
                    # ACT lane first (two-stage af+cand is the pair's long pole)
                    for li, b in ((1, 2 * p + 1), (0, 2 * p)):
                        ncol = hist_sb[:, (t - 1) * BL + b:(t - 1) * BL + b + 1]
                        ecol = logits_sb[:, (t - 1) * BL + b:(t - 1) * BL + b + 1]
                        dst = cand[:, b * K:(b + 1) * K]
                        if li == 0:
                            if t == 1:
                                nc.vector.tensor_scalar_add(dst, trans_sb[:], ncol)
                            else:
                                nc.vector.tensor_scalar(dst, trans_sb[:], ncol, ecol,
                                                        op0=OP.add, op1=OP.add)
                        else:
                            if t == 1:
                                nc.scalar.activation(dst, trans_sb[:], AF.Identity,
                                                     bias=ncol)
                            else:
                                afc = afp.tile([K, 1], f32, tag=f"af{b}")
                                nc.scalar.activation(afc[:], ncol, AF.Identity, bias=ecol)
                                nc.scalar.activation(dst, trans_sb[:], AF.Identity,
                                                     bias=afc[:])
                        nc.tensor.matmul(ct[:, li * K:(li + 1) * K], dst, ident_sb[:],
                                         is_transpose=True, start=True, stop=True)
                    nc.vector.tensor_reduce(hist_sb[:, t * BL + 2 * p:t * BL + 2 * p + 2],
                                            ct[:].rearrange("p (b i) -> p b i", b=2),
                                            axis=mybir.AxisListType.X, op=OP.max)
                # ---- forward ----
                u = ups.tile([K, BL], f32, tag="u")
                nc.tensor.matmul(u[:], etil_sb[:], pp, start=True, stop=True)
                ppn = ppp.tile([K, BL], bf16, tag="pp")
                nc.vector.tensor_tensor(ppn[:], u[:], eemz3[:, t, :], op=OP.mult)
                pp = ppn[:]
                if t % RENORM == 0 and t <= T - 3:
                    s = rnps.tile([1, BL], f32, tag="s")
                    nc.tensor.matmul(s[:], ones128_sb[:], pp, start=True, stop=True)
                    nc.vector.tensor_copy(stash_sb[0:1, stash_idx * BL:(stash_idx + 1) * BL], s[:])
                    r = ppp.tile([1, BL], f32, tag="r")
                    nc.vector.reciprocal(r[:], s[:])
                    rb = rnps.tile([K, BL], f32, tag="rb")
                    nc.tensor.matmul(rb[:], ones1_sb[:], r[:], start=True, stop=True)
                    pend_r = rb
                    stash_idx += 1
                elif t % RENORM == 2 and pend_r is not None:
                    ppn2 = ppp.tile([K, BL], bf16, tag="pp")
                    nc.vector.tensor_tensor(ppn2[:], pend_r[:], pp, op=OP.mult)
                    pp = ppn2[:]
                    pend_r = None
            nc.sync.dma_start(stash_out[:], stash_sb[:])
            ppfin_f32 = ppp.tile([K, BL], f32, tag="ppf")
            nc.vector.tensor_copy(ppfin_f32[:], pp)
            nc.sync.dma_start(ppfin_out[:], ppfin_f32[:])
        nc.sync.dma_start(hist_out[:], hist_sb[:])

    nc.compile()
    return nc


def _get_built():
    global _BUILT
    if _BUILT is None:
        _BUILT = _build()
    return _BUILT


def _reference_numpy(hiddens, mask, labels, W, b, start_t, end_t, trans):
    """Pure-numpy fallback replica of the reference (used only for unexpected masks)."""
    Bn, Tn, _ = hiddens.shape
    Kn = W.shape[1]
    logits = (hiddens.reshape(Bn * Tn, -1) @ W + b).reshape(Bn, Tn, Kn).astype(np.float32)
    maskf = mask.astype(np.float32)
    bi = np.arange(Bn)
    score = start_t[labels[:, 0]] + logits[bi, 0, labels[:, 0]]
    prev, cur = labels[:, :-1], labels[:, 1:]
    emit_cur = np.take_along_axis(logits[:, 1:], cur[..., None], axis=-1)[..., 0]
    score = score + ((trans[prev, cur] + emit_cur) * maskf[:, 1:]).sum(axis=1)
    last_idx = mask.sum(axis=1).astype(np.int64) - 1
    last_tags = np.take_along_axis(labels, last_idx[:, None], axis=1)[:, 0]
    score = score + end_t[last_tags]
    alpha = start_t[None, :] + logits[:, 0]
    for t in range(1, Tn):
        x = alpha[:, :, None] + trans[None]
        m = x.max(axis=1)
        nxt = m + np.log(np.exp(x - m[:, None, :]).sum(axis=1)) + logits[:, t]
        on = mask[:, t][:, None] > 0
        alpha = np.where(on, nxt, alpha)
    am = (alpha + end_t[None]).max(axis=1)
    logZ = am + np.log(np.exp(alpha + end_t[None] - am[:, None]).sum(axis=1))
    llh = score - logZ
    # viterbi
    tags_all = np.zeros((Bn, Tn), dtype=np.int32)
    score_v = start_t[None, :] + logits[:, 0]
    hist = np.zeros((Tn - 1, Bn, Kn), dtype=np.int32)
    for t in range(1, Tn):
        cand = score_v[:, :, None] + trans[None]
        bp = cand.argmax(axis=1).astype(np.int32)
        best = cand.max(axis=1) + logits[:, t]
        on = mask[:, t][:, None] > 0
        score_v = np.where(on, best, score_v)
        bp = np.where(on, bp, np.arange(Kn, dtype=np.int32)[None, :])
        hist[t - 1] = bp
    last = (score_v + end_t[None]).argmax(axis=1).astype(np.int32)
    tags_all[:, Tn - 1] = last
    cur_t = last
    for t in range(Tn - 2, -1, -1):
        cur_t = hist[t][bi, cur_t]
        tags_all[:, t] = cur_t
    loss = np.float32(-llh.mean())
    return tags_all, loss


def kernel(hiddens, mask, labels, W, b, start_transitions, end_transitions, transitions):
    hiddens = np.asarray(hiddens, dtype=np.float32)
    mask = np.asarray(mask).astype(np.int32)
    labels = np.asarray(labels).astype(np.int64)
    W_ = np.asarray(W, dtype=np.float32)
    b_ = np.asarray(b, dtype=np.float32)
    start_t = np.asarray(start_transitions, dtype=np.float32)
    end_t = np.asarray(end_transitions, dtype=np.float32)
    trans = np.asarray(transitions, dtype=np.float32)

    if not np.all(mask == 1):
        return _reference_numpy(hiddens, mask, labels, W_, b_, start_t, end_t, trans)

    from concourse.bass_utils import run_bass_kernel_spmd

    nc = _get_built()

    # host-side precompute
    hTall = np.ascontiguousarray(hiddens.transpose(0, 2, 1))  # [B, H, T]
    E = np.exp(trans.astype(np.float64))
    Z = E.sum(axis=1)                       # [K]
    Etil = (E / Z[:, None]).astype(ml_dtypes.bfloat16)
    lnZ = np.log(Z).astype(np.float32)
    cols = np.stack([b_, b_ + lnZ, b_ + start_t + lnZ, b_ + start_t], axis=1).astype(np.float32)
    ident = np.eye(K, dtype=np.float32)
    ones128 = np.ones((K, 1), dtype=ml_dtypes.bfloat16)
    ones1 = np.ones((1, K), dtype=np.float32)

    in_maps = []
    for c in range(N_CORES):
        in_maps.append({
            "hT": np.ascontiguousarray(hTall[c * BL:(c + 1) * BL]),
            "Wm": W_, "transM": trans, "EtilM": Etil, "identM": ident,
            "colsM": cols, "ones128M": ones128, "ones1M": ones1,
        })

    res = run_bass_kernel_spmd(nc, in_maps, core_ids=list(range(N_CORES)))

    # host-side assembly
    decode = np.zeros((B, T), dtype=np.int32)
    llh = np.zeros((B,), dtype=np.float64)
    bi = np.arange(BL)
    for c in range(N_CORES):
        r = res.results[c]
        logits = r["logits_out"].reshape(K, T, BL)     # [k, t, b]
        hist = r["hist_out"].reshape(K, T, BL).copy()  # ne for t>=1; alphaFull at t=0
        hist[:, 1:, :] += logits[:, 1:, :]             # -> alphaFull everywhere
        ppfin = r["ppfin_out"]                          # [K, BL]
        stash = r["stash_out"].reshape(N_STASH, BL)     # S values
        # logZ
        fin = (ppfin.astype(np.float64) / Z[:, None]) * np.exp(end_t.astype(np.float64))[:, None]
        logZ = np.log(stash.astype(np.float64)).sum(axis=0) + np.log(fin.sum(axis=0))  # [BL]
        # viterbi backtrace
        lab = labels[c * BL:(c + 1) * BL]
        tag = np.argmax(hist[:, T - 1, :] + end_t[:, None], axis=0)
        dec = np.zeros((BL, T), dtype=np.int32)
        dec[:, T - 1] = tag
        for t in range(T - 2, -1, -1):
            sc = hist[:, t, :] + trans[:, tag]          # [i, BL]
            tag = np.argmax(sc, axis=0)
            dec[:, t] = tag
        decode[c * BL:(c + 1) * BL] = dec
        # gold score
        lg = logits.transpose(2, 1, 0)                  # [BL, t, k]
        score = start_t[lab[:, 0]] + lg[bi, 0, lab[:, 0]]
        prev, cur = lab[:, :-1], lab[:, 1:]
        emit_cur = np.take_along_axis(lg[:, 1:], cur[..., None], axis=-1)[..., 0]
        score = score + (trans[prev, cur] + emit_cur).sum(axis=1)
        score = score + end_t[lab[:, -1]]
        llh[c * BL:(c + 1) * BL] = score - logZ

    loss = np.float32(-llh.mean())
    return decode, loss


# revision 24
# speedup vs baseline: 1.0660x; 1.0660x over previous
"""CRF decoder (projection + Viterbi decode + CRF log-likelihood loss) on 8 Trainium2 cores.

Strategy (data-parallel over batch, 4 batch elements per core):
 - host pre-transposes hiddens to [B, H, T] so the device can load [h, t] tiles
   contiguously; emission projection runs as fp32 PE matmuls (W chunks stationary).
 - forward (log-partition) scan runs in probability space: alpha_t in normalized
   prob form, transition mixing via one small bf16 PE matmul per step with a
   row-stochastic exp(transitions) matrix (row sums folded into the emission
   exponentials), periodic renormalization; log-corrections summed on host.
 - Viterbi max scan runs in fp32: per-step tensor_scalar candidate add (DVE+ACT),
   PE transposes, fused DVE max-reduce; the alpha history ships to DRAM and the
   (cheap, O(B*T*K)) backtrace runs on host, identical in exact arithmetic to the
   reference's backpointer path.
 - gold-path score and the final loss reduction are O(B*T) gathers on host.
"""

import os
import sys

for _p in ("/opt/trn_rl_repo", "/root/.axon_site/_ro/trn_rl_repo"):
    if os.path.isdir(_p) and _p not in sys.path:
        sys.path.append(_p)

from contextlib import ExitStack

import ml_dtypes
import numpy as np

B, T, H, K = 32, 512, 2048, 128
N_CORES = 8
BL = B // N_CORES          # batch per core
RENORM = 6                 # forward renorm interval (measure at t%R==0, apply at t%R==2)
N_STASH = (T - 3) // RENORM  # renorm measurements (t = R, 2R, ..., <= T-3)

_BUILT = None


def _build():
    import concourse.bass as bass
    import concourse.tile as tile
    from concourse import bacc, mybir

    f32 = mybir.dt.float32
    bf16 = mybir.dt.bfloat16
    AF = mybir.ActivationFunctionType
    OP = mybir.AluOpType

    nc = bacc.Bacc("TRN2", target_bir_lowering=False, debug=False, num_devices=N_CORES)

    # inputs
    hT = nc.dram_tensor("hT", [BL, H, T], f32, kind="ExternalInput").ap()
    Wm = nc.dram_tensor("Wm", [H, K], f32, kind="ExternalInput").ap()
    transM = nc.dram_tensor("transM", [K, K], f32, kind="ExternalInput").ap()
    EtilM = nc.dram_tensor("EtilM", [K, K], bf16, kind="ExternalInput").ap()
    identM = nc.dram_tensor("identM", [K, K], f32, kind="ExternalInput").ap()
    # bias columns: [K, 4] = (biasv, biaslnZ, pp0bias, v0bias)
    colsM = nc.dram_tensor("colsM", [K, 4], f32, kind="ExternalInput").ap()
    ones128M = nc.dram_tensor("ones128M", [K, 1], bf16, kind="ExternalInput").ap()
    ones1M = nc.dram_tensor("ones1M", [1, K], f32, kind="ExternalInput").ap()

    # outputs
    logits_out = nc.dram_tensor("logits_out", [K, T * BL], f32, kind="ExternalOutput").ap()
    hist_out = nc.dram_tensor("hist_out", [K, T * BL], f32, kind="ExternalOutput").ap()
    ppfin_out = nc.dram_tensor("ppfin_out", [K, BL], f32, kind="ExternalOutput").ap()
    stash_out = nc.dram_tensor("stash_out", [1, N_STASH * BL], f32, kind="ExternalOutput").ap()

    with tile.TileContext(nc) as tc, ExitStack() as ctx:
        const = ctx.enter_context(tc.tile_pool(name="const", bufs=1))
        big = ctx.enter_context(tc.tile_pool(name="big", bufs=1))

        w_sb = const.tile([K, H], f32)           # 16 W chunks, chunk h at cols [h*128,(h+1)*128)
        trans_sb = const.tile([K, K], f32)
        etil_sb = const.tile([K, K], bf16)
        ident_sb = const.tile([K, K], f32)
        cols_sb = const.tile([K, 4], f32)
        ones128_sb = const.tile([K, 1], bf16)
        ones1_sb = const.tile([1, K], f32)

        logits_sb = big.tile([K, T * BL], f32)   # col = t*BL + b
        eemz_sb = big.tile([K, T * BL], f32)     # exp(logits + lnZ), same layout
        hist_sb = big.tile([K, T * BL], f32)     # viterbi alpha history
        stash_sb = big.tile([1, N_STASH * BL], f32)
        pp0_sb = big.tile([K, BL], bf16)

        # W is [H, K] in DRAM; chunk h is rows [h*128,(h+1)*128) -> SBUF [128(h'), 128(k)]
        for h in range(H // K):
            nc.sync.dma_start(w_sb[:, h * K:(h + 1) * K], Wm[h * K:(h + 1) * K, :])
        nc.sync.dma_start(trans_sb[:], transM[:])
        nc.sync.dma_start(etil_sb[:], EtilM[:])
        nc.sync.dma_start(ident_sb[:], identM[:])
        nc.sync.dma_start(cols_sb[:], colsM[:])
        nc.sync.dma_start(ones128_sb[:], ones128M[:])
        nc.sync.dma_start(ones1_sb[:], ones1M[:])

        biasv = cols_sb[:, 0:1]
        biaslnZ = cols_sb[:, 1:2]
        pp0bias = cols_sb[:, 2:3]
        v0bias = cols_sb[:, 3:4]

        logits3 = logits_sb.rearrange("p (t b) -> p t b", b=BL)
        eemz3 = eemz_sb.rearrange("p (t b) -> p t b", b=BL)

        # ---------------- phase 1: emission projection ----------------
        with tc.tile_pool(name="hbuf", bufs=3) as hbuf, \
             tc.tile_pool(name="ppsum", bufs=2, space="PSUM") as ppsum:
            for b in range(BL):
                acc = ppsum.tile([K, T], f32)
                for h in range(H // K):
                    rhs = hbuf.tile([K, T], f32)
                    nc.sync.dma_start(rhs[:], hT[b, h * K:(h + 1) * K, :])
                    nc.tensor.matmul(acc[:], w_sb[:, h * K:(h + 1) * K], rhs[:],
                                     start=(h == 0), stop=(h == H // K - 1))
                # logits (+bias) into strided [:, :, b] columns
                nc.vector.tensor_scalar_add(logits3[:, :, b], acc[:], biasv)
                # exp(logits + lnZ) = exp(acc + bias + lnZ)
                nc.scalar.activation(eemz3[:, :, b], acc[:], AF.Exp, bias=biaslnZ)
                # forward init pp0 = exp(acc[:,0] + bias + start + lnZ)
                nc.scalar.activation(pp0_sb[:, b:b + 1], acc[:, 0:1], AF.Exp, bias=pp0bias)
                # viterbi init alpha0 = acc[:,0] + bias + start
                nc.vector.tensor_scalar_add(hist_sb[:, b:b + 1], acc[:, 0:1], v0bias)
        nc.sync.dma_start(logits_out[:], logits_sb[:])

        # -------- phase 2: fused forward scan (prob space) + viterbi max scan --------
        # forward: 1 chain, delayed renorm (measure at t%R==0, apply at t%R==2) so the
        #   renorm pipeline stays off the serial path.
        # viterbi: two independent half-phase pair-chains (lanes 01 / 23); per pair:
        #   DVE double-scalar cand + ACT two-stage cand, PE transpose-mode, own PSUM
        #   bank, own half-reduce writing alphaNoEmit into hist (host adds logits).
        with tc.tile_pool(name="ppp", bufs=3) as ppp, \
             tc.tile_pool(name="cand", bufs=3) as candp, \
             tc.tile_pool(name="af", bufs=3) as afp, \
             tc.tile_pool(name="ups", bufs=2, space="PSUM") as ups, \
             tc.tile_pool(name="rnps", bufs=1, space="PSUM") as rnps, \
             tc.tile_pool(name="ctps", bufs=2, space="PSUM") as ctps:
            pp = pp0_sb[:]
            af_prev = None
            stash_idx = 0
            pend_r = None
            # ~4us of normal-mode matmuls to flip the PE HAM clock gate to 8/8
            # before the scan; the forward's per-step matmul then keeps every
            # activity window non-idle so the PE never re-throttles.
            warm_ps = ctps.tile([K, 2 * K], f32, tag="ct0")
            for w in range(36):
                nc.tensor.matmul(warm_ps[:, 0:K], etil_sb[:], etil_sb[:],
                                 start=True, stop=True)
            for t in range(1, T):
                # ---- viterbi: two independent pair-chains, half-phase shifted ----
                # pair p covers lanes (2p, 2p+1): lane 2p cand on DVE (STT fuses ne+emit),
                # lane 2p+1 on ACT (ACT-side af then cand); each pair has its own PSUM
                # bank and its own half-reduce, so the chains never wait on each other.
                cand = candp.tile([K, BL * K], f32, tag="cand")
                for p in range(2):
                    ct = ctps.tile([K, 2 * K], f32, tag=f"ct{p}")
                    # ACT lane first (two-stage af+cand is the pair's long pole)
                    for li, b in ((1, 2 * p + 1), (0, 2 * p)):
                        ncol = hist_sb[:, (t - 1) * BL + b:(t - 1) * BL + b + 1]
                        ecol = logits_sb[:, (t - 1) * BL + b:(t - 1) * BL + b + 1]
                        dst = cand[:, b * K:(b + 1) * K]
                        if li == 0:
                            if t == 1:
                                nc.vector.tensor_scalar_add(dst, trans_sb[:], ncol)
                            else:
                                nc.vector.tensor_scalar(dst, trans_sb[:], ncol, ecol,
                                                        op0=OP.add, op1=OP.add)
                        else:
                            if t == 1:
                                nc.scalar.activation(dst, trans_sb[:], AF.Identity,
                                                     bias=ncol)
                            else:
                                afc = afp.tile([K, 1], f32, tag=f"af{b}")
                                nc.scalar.activation(afc[:], ncol, AF.Identity, bias=ecol)
                                nc.scalar.activation(dst, trans_sb[:], AF.Identity,
                                                     bias=afc[:])
                        nc.tensor.matmul(ct[:, li * K:(li + 1) * K], dst, ident_sb[:],
                                         is_transpose=True, start=True, stop=True)
                    nc.vector.tensor_reduce(hist_sb[:, t * BL + 2 * p:t * BL + 2 * p + 2],
                                            ct[:].rearrange("p (b i) -> p b i", b=2),
                                            axis=mybir.AxisListType.X, op=OP.max)
                # ---- forward ----
                u = ups.tile([K, BL], f32, tag="u")
                nc.tensor.matmul(u[:], etil_sb[:], pp, start=True, stop=True)
                ppn = ppp.tile([K, BL], bf16, tag="pp")
                nc.vector.tensor_tensor(ppn[:], u[:], eemz3[:, t, :], op=OP.mult)
                pp = ppn[:]
                if t % RENORM == 0 and t <= T - 3:
                    s = rnps.tile([1, BL], f32, tag="s")
                    nc.tensor.matmul(s[:], ones128_sb[:], pp, start=True, stop=True)
                    nc.vector.tensor_copy(stash_sb[0:1, stash_idx * BL:(stash_idx + 1) * BL], s[:])
                    r = ppp.tile([1, BL], f32, tag="r")
                    nc.vector.reciprocal(r[:], s[:])
                    rb = rnps.tile([K, BL], f32, tag="rb")
                    nc.tensor.matmul(rb[:], ones1_sb[:], r[:], start=True, stop=True)
                    pend_r = rb
                    stash_idx += 1
                elif t % RENORM == 2 and pend_r is not None:
                    ppn2 = ppp.tile([K, BL], bf16, tag="pp")
                    nc.vector.tensor_tensor(ppn2[:], pend_r[:], pp, op=OP.mult)
                    pp = ppn2[:]
                    pend_r = None
            nc.sync.dma_start(stash_out[:], stash_sb[:])
            ppfin_f32 = ppp.tile([K, BL], f32, tag="ppf")
            nc.vector.tensor_copy(ppfin_f32[:], pp)
            nc.sync.dma_start(ppfin_out[:], ppfin_f32[:])
        nc.sync.dma_start(hist_out[:], hist_sb[:])

    nc.compile()
    return nc


def _get_built():
    global _BUILT
    if _BUILT is None:
        _BUILT = _build()
    return _BUILT


def _reference_numpy(hiddens, mask, labels, W, b, start_t, end_t, trans):
    """Pure-numpy fallback replica of the reference (used only for unexpected masks)."""
    Bn, Tn, _ = hiddens.shape
    Kn = W.shape[1]
    logits = (hiddens.reshape(Bn * Tn, -1) @ W + b).reshape(Bn, Tn, Kn).astype(np.float32)
    maskf = mask.astype(np.float32)
    bi = np.arange(Bn)
    score = start_t[labels[:, 0]] + logits[bi, 0, labels[:, 0]]
    prev, cur = labels[:, :-1], labels[:, 1:]
    emit_cur = np.take_along_axis(logits[:, 1:], cur[..., None], axis=-1)[..., 0]
    score = score + ((trans[prev, cur] + emit_cur) * maskf[:, 1:]).sum(axis=1)
    last_idx = mask.sum(axis=1).astype(np.int64) - 1
    last_tags = np.take_along_axis(labels, last_idx[:, None], axis=1)[:, 0]
    score = score + end_t[last_tags]
    alpha = start_t[None, :] + logits[:, 0]
    for t in range(1, Tn):
        x = alpha[:, :, None] + trans[None]
        m = x.max(axis=1)
        nxt = m + np.log(np.exp(x - m[:, None, :]).sum(axis=1)) + logits[:, t]
        on = mask[:, t][:, None] > 0
        alpha = np.where(on, nxt, alpha)
    am = (alpha + end_t[None]).max(axis=1)
    logZ = am + np.log(np.exp(alpha + end_t[None] - am[:, None]).sum(axis=1))
    llh = score - logZ
    # viterbi
    tags_all = np.zeros((Bn, Tn), dtype=np.int32)
    score_v = start_t[None, :] + logits[:, 0]
    hist = np.zeros((Tn - 1, Bn, Kn), dtype=np.int32)
    for t in range(1, Tn):
        cand = score_v[:, :, None] + trans[None]
        bp = cand.argmax(axis=1).astype(np.int32)
        best = cand.max(axis=1) + logits[:, t]
        on = mask[:, t][:, None] > 0
        score_v = np.where(on, best, score_v)
        bp = np.where(on, bp, np.arange(Kn, dtype=np.int32)[None, :])
        hist[t - 1] = bp
    last = (score_v + end_t[None]).argmax(axis=1).astype(np.int32)
    tags_all[:, Tn - 1] = last
    cur_t = last
    for t in range(Tn - 2, -1, -1):
        cur_t = hist[t][bi, cur_t]
        tags_all[:, t] = cur_t
    loss = np.float32(-llh.mean())
    return tags_all, loss


def kernel(hiddens, mask, labels, W, b, start_transitions, end_transitions, transitions):
    hiddens = np.asarray(hiddens, dtype=np.float32)
    mask = np.asarray(mask).astype(np.int32)
    labels = np.asarray(labels).astype(np.int64)
    W_ = np.asarray(W, dtype=np.float32)
    b_ = np.asarray(b, dtype=np.float32)
    start_t = np.asarray(start_transitions, dtype=np.float32)
    end_t = np.asarray(end_transitions, dtype=np.float32)
    trans = np.asarray(transitions, dtype=np.float32)

    if not np.all(mask == 1):
        return _reference_numpy(hiddens, mask, labels, W_, b_, start_t, end_t, trans)

    from concourse.bass_utils import run_bass_kernel_spmd

    nc = _get_built()

    # host-side precompute
    hTall = np.ascontiguousarray(hiddens.transpose(0, 2, 1))  # [B, H, T]
    E = np.exp(trans.astype(np.float64))
    Z = E.sum(axis=1)                       # [K]
    Etil = (E / Z[:, None]).astype(ml_dtypes.bfloat16)
    lnZ = np.log(Z).astype(np.float32)
    cols = np.stack([b_, b_ + lnZ, b_ + start_t + lnZ, b_ + start_t], axis=1).astype(np.float32)
    ident = np.eye(K, dtype=np.float32)
    ones128 = np.ones((K, 1), dtype=ml_dtypes.bfloat16)
    ones1 = np.ones((1, K), dtype=np.float32)

    in_maps = []
    for c in range(N_CORES):
        in_maps.append({
            "hT": np.ascontiguousarray(hTall[c * BL:(c + 1) * BL]),
            "Wm": W_, "transM": trans, "EtilM": Etil, "identM": ident,
            "colsM": cols, "ones128M": ones128, "ones1M": ones1,
        })

    res = run_bass_kernel_spmd(nc, in_maps, core_ids=list(range(N_CORES)))

    # host-side assembly
    decode = np.zeros((B, T), dtype=np.int32)
    llh = np.zeros((B,), dtype=np.float64)
    bi = np.arange(BL)
    for c in range(N_CORES):
        r = res.results[c]
        logits = r["logits_out"].reshape(K, T, BL)     # [k, t, b]
        hist = r["hist_out"].reshape(K, T, BL).copy()  # ne for t>=1; alphaFull at t=0
        hist[:, 1:, :] += logits[:, 1:, :]             # -> alphaFull everywhere
        ppfin = r["ppfin_out"]                          # [K, BL]
        stash = r["stash_out"].reshape(N_STASH, BL)     # S values
        # logZ
        fin = (ppfin.astype(np.float64) / Z[:, None]) * np.exp(end_t.astype(np.float64))[:, None]
        logZ = np.log(stash.astype(np.float64)).sum(axis=0) + np.log(fin.sum(axis=0))  # [BL]
        # viterbi backtrace
        lab = labels[c * BL:(c + 1) * BL]
        tag = np.argmax(hist[:, T - 1, :] + end_t[:, None], axis=0)
        dec = np.zeros((BL, T), dtype=np.int32)
        dec[:, T - 1] = tag
        for t in range(T - 2, -1, -1):
            sc = hist[:, t, :] + trans[:, tag]          # [i, BL]
            tag = np.argmax(sc, axis=0)
            dec[:, t] = tag
        decode[c * BL:(c + 1) * BL] = dec
        # gold score
        lg = logits.transpose(2, 1, 0)                  # [BL, t, k]
        score = start_t[lab[:, 0]] + lg[bi, 0, lab[:, 0]]
        prev, cur = lab[:, :-1], lab[:, 1:]
        emit_cur = np.take_along_axis(lg[:, 1:], cur[..., None], axis=-1)[..., 0]
        score = score + (trans[prev, cur] + emit_cur).sum(axis=1)
        score = score + end_t[lab[:, -1]]
        llh[c * BL:(c + 1) * BL] = score - logZ

    loss = np.float32(-llh.mean())
    return decode, loss


# revision 26
# speedup vs baseline: 1.1266x; 1.0569x over previous
"""CRF decoder (projection + Viterbi decode + CRF log-likelihood loss) on 8 Trainium2 cores.

Strategy (data-parallel over batch, 4 batch elements per core):
 - host pre-transposes hiddens to [B, H, T] so the device can load [h, t] tiles
   contiguously; emission projection runs as fp32 PE matmuls (W chunks stationary).
 - forward (log-partition) scan runs in probability space: alpha_t in normalized
   prob form, transition mixing via one small bf16 PE matmul per step with a
   row-stochastic exp(transitions) matrix (row sums folded into the emission
   exponentials), periodic renormalization; log-corrections summed on host.
 - Viterbi max scan runs in fp32: per-step tensor_scalar candidate add (DVE+ACT),
   PE transposes, fused DVE max-reduce; the alpha history ships to DRAM and the
   (cheap, O(B*T*K)) backtrace runs on host, identical in exact arithmetic to the
   reference's backpointer path.
 - gold-path score and the final loss reduction are O(B*T) gathers on host.
"""

import os
import sys

for _p in ("/opt/trn_rl_repo", "/root/.axon_site/_ro/trn_rl_repo"):
    if os.path.isdir(_p) and _p not in sys.path:
        sys.path.append(_p)

from contextlib import ExitStack

import ml_dtypes
import numpy as np

B, T, H, K = 32, 512, 2048, 128
N_CORES = 8
BL = B // N_CORES          # batch per core
RENORM = 6                 # forward renorm interval (measure at t%R==0, apply at t%R==2)
N_STASH = (T - 3) // RENORM  # renorm measurements (t = R, 2R, ..., <= T-3)

_BUILT = None


def _build():
    import concourse.bass as bass
    import concourse.tile as tile
    from concourse import bacc, mybir

    f32 = mybir.dt.float32
    bf16 = mybir.dt.bfloat16
    AF = mybir.ActivationFunctionType
    OP = mybir.AluOpType

    nc = bacc.Bacc("TRN2", target_bir_lowering=False, debug=False, num_devices=N_CORES)

    # inputs
    hT = nc.dram_tensor("hT", [BL, H, T], f32, kind="ExternalInput").ap()
    Wm = nc.dram_tensor("Wm", [H, K], f32, kind="ExternalInput").ap()
    transM = nc.dram_tensor("transM", [K, K], f32, kind="ExternalInput").ap()
    EtilM = nc.dram_tensor("EtilM", [K, K], bf16, kind="ExternalInput").ap()
    identM = nc.dram_tensor("identM", [K, K], f32, kind="ExternalInput").ap()
    # bias columns: [K, 4] = (biasv, biaslnZ, pp0bias, v0bias)
    colsM = nc.dram_tensor("colsM", [K, 4], f32, kind="ExternalInput").ap()
    ones128M = nc.dram_tensor("ones128M", [K, 1], bf16, kind="ExternalInput").ap()
    ones1M = nc.dram_tensor("ones1M", [1, K], f32, kind="ExternalInput").ap()

    # outputs
    logits_out = nc.dram_tensor("logits_out", [K, T * BL], f32, kind="ExternalOutput").ap()
    hist_out = nc.dram_tensor("hist_out", [K, T * BL], f32, kind="ExternalOutput").ap()
    ppfin_out = nc.dram_tensor("ppfin_out", [K, BL], f32, kind="ExternalOutput").ap()
    stash_out = nc.dram_tensor("stash_out", [1, N_STASH * BL], f32, kind="ExternalOutput").ap()

    with tile.TileContext(nc) as tc, ExitStack() as ctx:
        const = ctx.enter_context(tc.tile_pool(name="const", bufs=1))
        big = ctx.enter_context(tc.tile_pool(name="big", bufs=1))

        w_sb = const.tile([K, H], f32)           # 16 W chunks, chunk h at cols [h*128,(h+1)*128)
        trans_sb = const.tile([K, K], f32)
        etil_sb = const.tile([K, K], bf16)
        ident_sb = const.tile([K, K], f32)
        cols_sb = const.tile([K, 4], f32)
        ones128_sb = const.tile([K, 1], bf16)
        ones1_sb = const.tile([1, K], f32)

        logits_sb = big.tile([K, T * BL], f32)   # col = t*BL + b
        eemz_sb = big.tile([K, T * BL], f32)     # exp(logits + lnZ), same layout
        hist_sb = big.tile([K, T * BL], f32)     # viterbi alpha history
        stash_sb = big.tile([1, N_STASH * BL], f32)
        pp0_sb = big.tile([K, BL], bf16)

        # W is [H, K] in DRAM; chunk h is rows [h*128,(h+1)*128) -> SBUF [128(h'), 128(k)]
        for h in range(H // K):
            nc.sync.dma_start(w_sb[:, h * K:(h + 1) * K], Wm[h * K:(h + 1) * K, :])
        nc.sync.dma_start(trans_sb[:], transM[:])
        nc.sync.dma_start(etil_sb[:], EtilM[:])
        nc.sync.dma_start(ident_sb[:], identM[:])
        nc.sync.dma_start(cols_sb[:], colsM[:])
        nc.sync.dma_start(ones128_sb[:], ones128M[:])
        nc.sync.dma_start(ones1_sb[:], ones1M[:])

        biasv = cols_sb[:, 0:1]
        biaslnZ = cols_sb[:, 1:2]
        pp0bias = cols_sb[:, 2:3]
        v0bias = cols_sb[:, 3:4]

        logits3 = logits_sb.rearrange("p (t b) -> p t b", b=BL)
        eemz3 = eemz_sb.rearrange("p (t b) -> p t b", b=BL)

        # ---------------- phase 1: emission projection ----------------
        with tc.tile_pool(name="hbuf", bufs=8) as hbuf, \
             tc.tile_pool(name="ppsum", bufs=2, space="PSUM") as ppsum:
            for b in range(BL):
                acc = ppsum.tile([K, T], f32)
                for h in range(H // K):
                    rhs = hbuf.tile([K, T], f32)
                    nc.sync.dma_start(rhs[:], hT[b, h * K:(h + 1) * K, :])
                    nc.tensor.matmul(acc[:], w_sb[:, h * K:(h + 1) * K], rhs[:],
                                     start=(h == 0), stop=(h == H // K - 1))
                # logits (+bias) into strided [:, :, b] columns
                nc.vector.tensor_scalar_add(logits3[:, :, b], acc[:], biasv)
                # exp(logits + lnZ) = exp(acc + bias + lnZ)
                nc.scalar.activation(eemz3[:, :, b], acc[:], AF.Exp, bias=biaslnZ)
                # forward init pp0 = exp(acc[:,0] + bias + start + lnZ)
                nc.scalar.activation(pp0_sb[:, b:b + 1], acc[:, 0:1], AF.Exp, bias=pp0bias)
                # viterbi init alpha0 = acc[:,0] + bias + start
                nc.vector.tensor_scalar_add(hist_sb[:, b:b + 1], acc[:, 0:1], v0bias)
        nc.sync.dma_start(logits_out[:], logits_sb[:])

        # -------- phase 2: fused forward scan (prob space) + viterbi max scan --------
        # forward: 1 chain, delayed renorm (measure at t%R==0, apply at t%R==2) so the
        #   renorm pipeline stays off the serial path.
        # viterbi: two independent half-phase pair-chains (lanes 01 / 23); per pair:
        #   DVE double-scalar cand + ACT two-stage cand, PE transpose-mode, own PSUM
        #   bank, own half-reduce writing alphaNoEmit into hist (host adds logits).
        with tc.tile_pool(name="ppp", bufs=3) as ppp, \
             tc.tile_pool(name="cand", bufs=3) as candp, \
             tc.tile_pool(name="af", bufs=3) as afp, \
             tc.tile_pool(name="ups", bufs=2, space="PSUM") as ups, \
             tc.tile_pool(name="rnps", bufs=1, space="PSUM") as rnps, \
             tc.tile_pool(name="ctps", bufs=2, space="PSUM") as ctps:
            pp = pp0_sb[:]
            af_prev = None
            stash_idx = 0
            pend_r = None
            for t in range(1, T):
                # ---- viterbi: two independent pair-chains, half-phase shifted ----
                # pair p covers lanes (2p, 2p+1): lane 2p cand on DVE (STT fuses ne+emit),
                # lane 2p+1 on ACT (ACT-side af then cand); each pair has its own PSUM
                # bank and its own half-reduce, so the chains never wait on each other.
                cand = candp.tile([K, BL * K], f32, tag="cand")
                for p in range(2):
                    ct = ctps.tile([K, 2 * K], f32, tag=f"ct{p}")
                    # ACT lane first (two-stage af+cand is the pair's long pole)
                    for li, b in ((1, 2 * p + 1), (0, 2 * p)):
                        ncol = hist_sb[:, (t - 1) * BL + b:(t - 1) * BL + b + 1]
                        ecol = logits_sb[:, (t - 1) * BL + b:(t - 1) * BL + b + 1]
                        dst = cand[:, b * K:(b + 1) * K]
                        if li == 0:
                            if t == 1:
                                nc.vector.tensor_scalar_add(dst, trans_sb[:], ncol)
                            else:
                                nc.vector.tensor_scalar(dst, trans_sb[:], ncol, ecol,
                                                        op0=OP.add, op1=OP.add)
                        else:
                            if t == 1:
                                nc.scalar.activation(dst, trans_sb[:], AF.Identity,
                                                     bias=ncol)
                            else:
                                afc = afp.tile([K, 1], f32, tag=f"af{b}")
                                nc.scalar.activation(afc[:], ncol, AF.Identity, bias=ecol)
                                nc.scalar.activation(dst, trans_sb[:], AF.Identity,
                                                     bias=afc[:])
                        nc.tensor.matmul(ct[:, li * K:(li + 1) * K], dst, ident_sb[:],
                                         is_transpose=True, start=True, stop=True)
                    nc.vector.tensor_reduce(hist_sb[:, t * BL + 2 * p:t * BL + 2 * p + 2],
                                            ct[:].rearrange("p (b i) -> p b i", b=2),
                                            axis=mybir.AxisListType.X, op=OP.max)
                # ---- forward ----
                u = ups.tile([K, BL], f32, tag="u")
                nc.tensor.matmul(u[:], etil_sb[:], pp, start=True, stop=True)
                ppn = ppp.tile([K, BL], bf16, tag="pp")
                nc.vector.tensor_tensor(ppn[:], u[:], eemz3[:, t, :], op=OP.mult)
                pp = ppn[:]
                if t % RENORM == 0 and t <= T - 3:
                    s = rnps.tile([1, BL], f32, tag="s")
                    nc.tensor.matmul(s[:], ones128_sb[:], pp, start=True, stop=True)
                    nc.vector.tensor_copy(stash_sb[0:1, stash_idx * BL:(stash_idx + 1) * BL], s[:])
                    r = ppp.tile([1, BL], f32, tag="r")
                    nc.vector.reciprocal(r[:], s[:])
                    rb = rnps.tile([K, BL], f32, tag="rb")
                    nc.tensor.matmul(rb[:], ones1_sb[:], r[:], start=True, stop=True)
                    pend_r = rb
                    stash_idx += 1
                elif t % RENORM == 2 and pend_r is not None:
                    ppn2 = ppp.tile([K, BL], bf16, tag="pp")
                    nc.vector.tensor_tensor(ppn2[:], pend_r[:], pp, op=OP.mult)
                    pp = ppn2[:]
                    pend_r = None
            nc.sync.dma_start(stash_out[:], stash_sb[:])
            ppfin_f32 = ppp.tile([K, BL], f32, tag="ppf")
            nc.vector.tensor_copy(ppfin_f32[:], pp)
            nc.sync.dma_start(ppfin_out[:], ppfin_f32[:])
        nc.sync.dma_start(hist_out[:], hist_sb[:])

    nc.compile()
    return nc


def _get_built():
    global _BUILT
    if _BUILT is None:
        _BUILT = _build()
    return _BUILT


def _reference_numpy(hiddens, mask, labels, W, b, start_t, end_t, trans):
    """Pure-numpy fallback replica of the reference (used only for unexpected masks)."""
    Bn, Tn, _ = hiddens.shape
    Kn = W.shape[1]
    logits = (hiddens.reshape(Bn * Tn, -1) @ W + b).reshape(Bn, Tn, Kn).astype(np.float32)
    maskf = mask.astype(np.float32)
    bi = np.arange(Bn)
    score = start_t[labels[:, 0]] + logits[bi, 0, labels[:, 0]]
    prev, cur = labels[:, :-1], labels[:, 1:]
    emit_cur = np.take_along_axis(logits[:, 1:], cur[..., None], axis=-1)[..., 0]
    score = score + ((trans[prev, cur] + emit_cur) * maskf[:, 1:]).sum(axis=1)
    last_idx = mask.sum(axis=1).astype(np.int64) - 1
    last_tags = np.take_along_axis(labels, last_idx[:, None], axis=1)[:, 0]
    score = score + end_t[last_tags]
    alpha = start_t[None, :] + logits[:, 0]
    for t in range(1, Tn):
        x = alpha[:, :, None] + trans[None]
        m = x.max(axis=1)
        nxt = m + np.log(np.exp(x - m[:, None, :]).sum(axis=1)) + logits[:, t]
        on = mask[:, t][:, None] > 0
        alpha = np.where(on, nxt, alpha)
    am = (alpha + end_t[None]).max(axis=1)
    logZ = am + np.log(np.exp(alpha + end_t[None] - am[:, None]).sum(axis=1))
    llh = score - logZ
    # viterbi
    tags_all = np.zeros((Bn, Tn), dtype=np.int32)
    score_v = start_t[None, :] + logits[:, 0]
    hist = np.zeros((Tn - 1, Bn, Kn), dtype=np.int32)
    for t in range(1, Tn):
        cand = score_v[:, :, None] + trans[None]
        bp = cand.argmax(axis=1).astype(np.int32)
        best = cand.max(axis=1) + logits[:, t]
        on = mask[:, t][:, None] > 0
        score_v = np.where(on, best, score_v)
        bp = np.where(on, bp, np.arange(Kn, dtype=np.int32)[None, :])
        hist[t - 1] = bp
    last = (score_v + end_t[None]).argmax(axis=1).astype(np.int32)
    tags_all[:, Tn - 1] = last
    cur_t = last
    for t in range(Tn - 2, -1, -1):
        cur_t = hist[t][bi, cur_t]
        tags_all[:, t] = cur_t
    loss = np.float32(-llh.mean())
    return tags_all, loss


def kernel(hiddens, mask, labels, W, b, start_transitions, end_transitions, transitions):
    hiddens = np.asarray(hiddens, dtype=np.float32)
    mask = np.asarray(mask).astype(np.int32)
    labels = np.asarray(labels).astype(np.int64)
    W_ = np.asarray(W, dtype=np.float32)
    b_ = np.asarray(b, dtype=np.float32)
    start_t = np.asarray(start_transitions, dtype=np.float32)
    end_t = np.asarray(end_transitions, dtype=np.float32)
    trans = np.asarray(transitions, dtype=np.float32)

    if not np.all(mask == 1):
        return _reference_numpy(hiddens, mask, labels, W_, b_, start_t, end_t, trans)

    from concourse.bass_utils import run_bass_kernel_spmd

    nc = _get_built()

    # host-side precompute
    hTall = np.ascontiguousarray(hiddens.transpose(0, 2, 1))  # [B, H, T]
    E = np.exp(trans.astype(np.float64))
    Z = E.sum(axis=1)                       # [K]
    Etil = (E / Z[:, None]).astype(ml_dtypes.bfloat16)
    lnZ = np.log(Z).astype(np.float32)
    cols = np.stack([b_, b_ + lnZ, b_ + start_t + lnZ, b_ + start_t], axis=1).astype(np.float32)
    ident = np.eye(K, dtype=np.float32)
    ones128 = np.ones((K, 1), dtype=ml_dtypes.bfloat16)
    ones1 = np.ones((1, K), dtype=np.float32)

    in_maps = []
    for c in range(N_CORES):
        in_maps.append({
            "hT": np.ascontiguousarray(hTall[c * BL:(c + 1) * BL]),
            "Wm": W_, "transM": trans, "EtilM": Etil, "identM": ident,
            "colsM": cols, "ones128M": ones128, "ones1M": ones1,
        })

    res = run_bass_kernel_spmd(nc, in_maps, core_ids=list(range(N_CORES)))

    # host-side assembly
    decode = np.zeros((B, T), dtype=np.int32)
    llh = np.zeros((B,), dtype=np.float64)
    bi = np.arange(BL)
    for c in range(N_CORES):
        r = res.results[c]
        logits = r["logits_out"].reshape(K, T, BL)     # [k, t, b]
        hist = r["hist_out"].reshape(K, T, BL).copy()  # ne for t>=1; alphaFull at t=0
        hist[:, 1:, :] += logits[:, 1:, :]             # -> alphaFull everywhere
        ppfin = r["ppfin_out"]                          # [K, BL]
        stash = r["stash_out"].reshape(N_STASH, BL)     # S values
        # logZ
        fin = (ppfin.astype(np.float64) / Z[:, None]) * np.exp(end_t.astype(np.float64))[:, None]
        logZ = np.log(stash.astype(np.float64)).sum(axis=0) + np.log(fin.sum(axis=0))  # [BL]
        # viterbi backtrace
        lab = labels[c * BL:(c + 1) * BL]
        tag = np.argmax(hist[:, T - 1, :] + end_t[:, None], axis=0)
        dec = np.zeros((BL, T), dtype=np.int32)
        dec[:, T - 1] = tag
        for t in range(T - 2, -1, -1):
            sc = hist[:, t, :] + trans[:, tag]          # [i, BL]
            tag = np.argmax(sc, axis=0)
            dec[:, t] = tag
        decode[c * BL:(c + 1) * BL] = dec
        # gold score
        lg = logits.transpose(2, 1, 0)                  # [BL, t, k]
        score = start_t[lab[:, 0]] + lg[bi, 0, lab[:, 0]]
        prev, cur = lab[:, :-1], lab[:, 1:]
        emit_cur = np.take_along_axis(lg[:, 1:], cur[..., None], axis=-1)[..., 0]
        score = score + (trans[prev, cur] + emit_cur).sum(axis=1)
        score = score + end_t[lab[:, -1]]
        llh[c * BL:(c + 1) * BL] = score - logZ

    loss = np.float32(-llh.mean())
    return decode, loss
